# revision 29
# baseline (speedup 1.0000x reference)
"""DisorderedCausalSelfAttention on 8 Trainium2 NeuronCores.

Problem: y = proj(causal_attn(rope_bias(qkv(x)))) with
  B=2, T=2048, C=1024, NH=16, D=64, RD=32 (partial RoPE), per-head
  additive biases bQ/bK applied post-RoPE.

Sharding: core c -> (batch b = c//4, head-group g = c%4 of 4 heads).
Each core computes qkv for its 4 heads, attention, and a partial output
projection (its 256 rows of W_proj); the host sums the 4 partials per
batch and adds b_proj.

v2 design (vs the phase-serial f32r v1):
  - bf16 storage for every matmul operand (same PE rate as f32r, half
    the DMA traffic and SBUF, no fp32r small-moving-dim penalty, DVE
    2x/4x modes); PSUM accumulation stays f32.
  - one fused pipeline over t-slices of 512 positions: qkv projection +
    RoPE for slice t, then causal attention rows qt=t for both head
    pairs, then the output projection columns of slice t with its DMA
    out -- so input streaming, compute, and output write-back overlap.
  - PSUM: a single 2-bank ring (bufs=2) carries proj pairs, rope-perm
    pairs, V quads, S tiles and out-proj pairs; ys accumulators get
    2x2 banks.  Exactly 8 banks.
  - S matmuls are causally trimmed (columns >= c0 of the q-tile).
  - psum->sbuf copies ride Act; masks/rope/normalize/out copies on DVE.
"""

import sys

sys.path.insert(0, "/opt/trn_rl_repo")

import json

import numpy as np
from ml_dtypes import bfloat16

B, T, C, NH, D, RD = 2, 2048, 1024, 16, 64, 32
G = 4  # head-groups (cores per batch)
HPG = NH // G  # heads per group = 4
N_CORES = 8
SCALE = float(D) ** -0.5
NT = T // 512  # 4 t-slices
NK = T // 128  # 16 k tiles

_cache = {}


# ---------------------------------------------------------------------------
# Workaround: this container's walrus build accepts at most ONE sync-wait
# command on most instructions, while Tile emits up to ~4.  Split excess
# waits into EventSemaphore instructions inserted immediately before, on the
# same engine (same-queue program order keeps semantics).
# ---------------------------------------------------------------------------
def _split_waits(bj: bytes, es_cap: int = 2) -> bytes:
    d = json.loads(bj)
    for fn in d.get("functions", []):
        for bb in fn.get("blocks", []):
            new = []
            for inst in bb.get("instructions", []):
                si = inst.get("sync_info") or {}
                w = si.get("on_wait") or []
                lim = es_cap if inst.get("opcode") == "EventSemaphore" else 1
                if len(w) > lim:
                    keep = w[-lim:]
                    mv = w[:-lim]
                    for ci in range(0, len(mv), es_cap):
                        new.append({
                            "debug": inst.get("debug"),
                            "engine": inst["engine"],
                            "ins": [], "outs": [],
                            "name": f"{inst['name']}_ws{ci}",
                            "opcode": "EventSemaphore",
                            "sync_info": {"on_update": [],
                                          "on_wait": mv[ci:ci + es_cap]},
                        })
                    si["on_wait"] = keep
                new.append(inst)
            bb["instructions"] = new
    return json.dumps(d).encode()


def _install_waitsplit():
    from concourse import bass2jax, bass_utils

    if getattr(bass2jax.compile_bir_kernel, "_waitsplit", False):
        return
    orig = bass_utils.compile_bir_kernel

    def patched(bj, tmpdir, neff_name="file.neff"):
        return orig(_split_waits(bj), tmpdir, neff_name)

    patched._waitsplit = True
    bass2jax.compile_bir_kernel = patched


# ---------------------------------------------------------------------------
# Kernel builder (one SPMD program; per-core data differs via in_maps)
# ---------------------------------------------------------------------------
def _build(loop_k: int = 1):
    import concourse.bass as bass
    import concourse.tile as tile
    from concourse import mybir

    f32 = mybir.dt.float32
    bf16 = mybir.dt.bfloat16
    Exp = mybir.ActivationFunctionType.Exp

    nc = bass.Bass("TRN2")

    # DRAM parameters, host-packed to match SBUF layouts exactly.
    xt = nc.declare_dram_parameter("x_t", [128, NT, 8, 512], bf16, isOutput=False)
    wqk = nc.declare_dram_parameter("w_qk", [128, 4, 8, 128], bf16, isOutput=False)
    wv = nc.declare_dram_parameter("w_v", [128, 8, 256], bf16, isOutput=False)
    wp = nc.declare_dram_parameter("w_p", [128, 2, 1024], bf16, isOutput=False)
    cosr = nc.declare_dram_parameter("cos_r", [128, T], bf16, isOutput=False)
    sinr = nc.declare_dram_parameter("sin_r", [128, T], bf16, isOutput=False)
    bqk = nc.declare_dram_parameter("bias_qk", [128, 4], f32, isOutput=False)
    trim = nc.declare_dram_parameter("tri", [128, 128], bf16, isOutput=False)
    perm = nc.declare_dram_parameter("perm", [128, 128], bf16, isOutput=False)
    out = nc.declare_dram_parameter("out", [T, C], bf16, isOutput=True)

    with tile.TileContext(nc) as tc:
      for _rep in range(loop_k):
        with tc.tile_pool(name="persist", bufs=1) as pp:
            XT = pp.tile([128, NT, 8, 512], bf16)
            WQK = pp.tile([128, 4, 8, 128], bf16)   # chunk order q01,k01,q23,k23
            WV = pp.tile([128, 8, 256], bf16)
            WP = pp.tile([128, 2, 1024], bf16)
            COS = pp.tile([128, T], bf16)
            SIN = pp.tile([128, T], bf16)
            BQK = pp.tile([128, 4], f32)
            TRI = pp.tile([128, 128], bf16)
            PERM = pp.tile([128, 128], bf16)
            QK = pp.tile([128, 4, T], bf16)          # chunks q01,k01,q23,k23
            V4 = pp.tile([128, NK, HPG, 2 * D], bf16)
            YT = pp.tile([128, 2, T], bf16)

            # weights on the SWDGE (gpsimd) queue in consumption order;
            # first chunk split small so the first matmul can start early.
            nc.sync.dma_start(out=WQK[:, 0:1], in_=wqk[:, 0:1])
            nc.gpsimd.dma_start(out=WQK[:, 1:2], in_=wqk[:, 1:2])
            nc.gpsimd.dma_start(out=WQK[:, 2:4], in_=wqk[:, 2:4])
            nc.gpsimd.dma_start(out=WV, in_=wv[:, :, :])
            nc.gpsimd.dma_start(out=WP, in_=wp[:, :, :])
            nc.gpsimd.memset(V4[:, :, :, D:], 1.0)

            def _rope_slices(t):
                sl = slice(t * 512, (t + 1) * 512)
                nc.sync.dma_start(out=COS[:, sl], in_=cosr[:, sl])
                nc.sync.dma_start(out=SIN[:, sl], in_=sinr[:, sl])

            nc.sync.dma_start(out=XT[:, 0, 0:2], in_=xt[:, 0, 0:2])
            nc.sync.dma_start(out=XT[:, 0, 2:8], in_=xt[:, 0, 2:8])
            _rope_slices(0)
            nc.sync.dma_start(out=BQK, in_=bqk[:, :])
            nc.sync.dma_start(out=TRI, in_=trim[:, :])
            nc.sync.dma_start(out=PERM, in_=perm[:, :])
            for t in range(1, NT):
                nc.sync.dma_start(out=XT[:, t], in_=xt[:, t])
                _rope_slices(t)

            with (
                tc.tile_pool(name="ps", bufs=2, space="PSUM") as ps,
                tc.tile_pool(name="sb", bufs=3) as sb,
            ):
                # ---- emission units ------------------------------------
                def proj_pair(t, mp, j, st):
                    """half of a qk-projection chunk pair for slice t."""
                    tc0, tc1 = t * 512, (t + 1) * 512
                    if j == 0:
                        st["pa"] = ps.tile([128, 2, 512], f32, tag="s", bufs=3,
                                           name=f"pa_{t}_{mp}")
                    pa = st["pa"]
                    for c in range(8):
                        nc.tensor.matmul(
                            pa[:, j, :],
                            WQK[:, 2 * mp + j, c, :],
                            XT[:, t, c, :],
                            start=(c == 0), stop=(c == 7),
                        )
                    if j == 1:
                        nc.vector.tensor_copy(
                            QK[:, 2 * mp:2 * mp + 2, tc0:tc1], pa)

                def rope_pair(t, mp):
                    """RoPE for chunk pair mp of slice t (after proj_pair)."""
                    tc0, tc1 = t * 512, (t + 1) * 512
                    pr = ps.tile([128, 2, 512], f32, tag="s", bufs=3,
                                 name=f"pr_{t}_{mp}")
                    for j in range(2):
                        nc.tensor.matmul(
                            pr[:, j, :], PERM,
                            QK[:, 2 * mp + j, tc0:tc1],
                            start=True, stop=True)
                    tmp = sb.tile([128, 2, 512], bf16, tag="tmp",
                                  name=f"tmp_{t}_{mp}")
                    nc.vector.tensor_mul(
                        tmp, pr,
                        SIN[:, None, tc0:tc1].broadcast_to([128, 2, 512]))
                    qsl = QK[:, 2 * mp:2 * mp + 2, tc0:tc1]
                    nc.vector.tensor_mul(
                        qsl, qsl,
                        COS[:, None, tc0:tc1].broadcast_to([128, 2, 512]))
                    for j in range(2):
                        m = 2 * mp + j
                        nc.vector.scalar_tensor_tensor(
                            QK[:, m, tc0:tc1], tmp[:, j, :], BQK[:, m:m + 1],
                            QK[:, m, tc0:tc1],
                            mybir.AluOpType.add, mybir.AluOpType.add)

                def v_pair(t, kq, j, st):
                    """half of a V-projection k-tile pair of slice t."""
                    if j == 0:
                        st["pv"] = ps.tile([128, 2, 512], f32, tag="s", bufs=3,
                                           name=f"pv_{t}_{kq}")
                    pv = st["pv"]
                    for c in range(8):
                        nc.tensor.matmul(
                            pv[:, j, 0:256],
                            XT[:, t, c, (2 * kq + j) * 128:
                               (2 * kq + j) * 128 + 128],
                            WV[:, c, :],
                            start=(c == 0), stop=(c == 7),
                        )
                    if j == 1:
                        nc.scalar.copy(
                            V4[:, 4 * t + 2 * kq:4 * t + 2 * kq + 2, :, 0:D],
                            pv[:, :, 0:256].rearrange(
                                "p j (h d) -> p j h d", h=HPG),
                        )

                def out_tile(tt, on_act=False):
                    """output projection for row-tile tt (128 rows)."""
                    po = ps.tile([128, 2, 512], f32, tag="s", bufs=3,
                                 name=f"po_{tt}")
                    for n in range(2):
                        for cc in range(2):
                            nc.tensor.matmul(
                                po[:, n, :],
                                YT[:, cc, tt * 128:(tt + 1) * 128],
                                WP[:, cc, n * 512:(n + 1) * 512],
                                start=(cc == 0), stop=(cc == 1),
                            )
                    ob = sb.tile([128, 1024], bf16, tag="ob",
                                 name=f"ob_{tt}")
                    obv = ob.rearrange("p (n w) -> p n w", n=2)
                    if on_act:
                        nc.scalar.copy(obv, po)
                    else:
                        nc.vector.tensor_copy(obv, po)
                    nc.sync.dma_start(out=out[tt * 128:(tt + 1) * 128, :],
                                      in_=ob)

                def proj_units(t):
                    stA, stB, svA, svB = {}, {}, {}, {}
                    yield lambda: proj_pair(t, 0, 0, stA)
                    yield lambda: proj_pair(t, 0, 1, stA)
                    yield lambda: rope_pair(t, 0)
                    yield lambda: v_pair(t, 0, 0, svA)
                    yield lambda: v_pair(t, 0, 1, svA)
                    yield lambda: proj_pair(t, 1, 0, stB)
                    yield lambda: proj_pair(t, 1, 1, stB)
                    yield lambda: rope_pair(t, 1)
                    yield lambda: v_pair(t, 1, 0, svB)
                    yield lambda: v_pair(t, 1, 1, svB)

                def attention(t, fillers, tail=False):
                    """Causal attention rows qt=t for both head pairs,
                    interleaving filler units (next slice's projection /
                    previous slice's output projection) into the PE stream
                    to hide exp/rope latency."""
                    tc0, tc1 = t * 512, (t + 1) * 512
                    nkt = 4 * t + 4
                    ngroups = 2 * nkt
                    nf = len(fillers)
                    fi = 0
                    gi = 0
                    for hp in range(2):
                        qc, kc = 2 * hp, 2 * hp + 1
                        ys = ps.tile([128, 2, 512], f32, tag="ys", bufs=1,
                                     name=f"ys_{t}_{hp}")
                        for kt in range(nkt):
                            j = kt - 4 * t
                            c0 = max(j, 0) * 128
                            s = ps.tile([128, 2, 512], f32, tag="s", bufs=3,
                                        name=f"s_{t}_{hp}_{kt}")
                            for hi in range(2):
                                o = hi * 64
                                nc.tensor.matmul(
                                    s[:, hi, c0:],
                                    QK[o:o + 64, kc, kt * 128:(kt + 1) * 128],
                                    QK[o:o + 64, qc, tc0 + c0:tc1],
                                    start=True, stop=True,
                                )
                            p = sb.tile([128, 2, 512], bf16, tag="p",
                                        bufs=4, name=f"p_{t}_{hp}_{kt}")
                            nc.scalar.activation(p[:, :, c0:], s[:, :, c0:],
                                                 Exp, scale=SCALE)
                            if j >= 0:
                                nc.vector.tensor_mul(
                                    p[:, :, c0:c0 + 128], p[:, :, c0:c0 + 128],
                                    TRI[:, None, :].broadcast_to([128, 2, 128]))
                            # filler between S/exp and AV hides exp latency
                            gi += 1
                            while fi < nf and fi + 1 <= (gi * nf) // ngroups:
                                fillers[fi]()
                                fi += 1
                            for hi in range(2):
                                nc.tensor.matmul(
                                    ys[:, hi, c0:],
                                    V4[:, kt, 2 * hp + hi, :],
                                    p[:, hi, c0:],
                                    start=(kt == 0), stop=(kt == nkt - 1),
                                )
                        # release ys with one wide copy to SBUF, then
                        # normalize off the critical path: rows 64:128 hold
                        # the softmax denominators (ones-block matmul).
                        if tail and hp == 1:
                            # last stretch: normalize straight from ys psum,
                            # per column, so each final out-proj tile starts
                            # as early as possible (nothing reuses ys after).
                            rb = sb.tile([64, 2, 512], f32, tag="rb",
                                         name=f"rb_{t}_{hp}")
                            for q in range(4):
                                ql, qh = q * 128, (q + 1) * 128
                                nc.vector.reciprocal(
                                    rb[:, :, ql:qh], ys[64:128, :, ql:qh])
                                for hi in range(2):
                                    o = hi * 64
                                    nc.vector.tensor_mul(
                                        YT[o:o + 64, hp, tc0 + ql:tc0 + qh],
                                        ys[0:D, hi, ql:qh],
                                        rb[:, hi, ql:qh])
                                out_tile(4 * t + q, q % 2 == 0)
                        else:
                            ysb = sb.tile([128, 2, 512], f32, tag="ysb",
                                          bufs=2, name=f"ysb_{t}_{hp}")
                            nc.vector.tensor_copy(ysb, ys)
                            rb = sb.tile([64, 2, 512], f32, tag="rb",
                                         name=f"rb_{t}_{hp}")
                            nc.vector.reciprocal(rb, ysb[64:128, :, :])
                            for hi in range(2):
                                o = hi * 64
                                nc.vector.tensor_mul(
                                    YT[o:o + 64, hp, tc0:tc1],
                                    ysb[0:D, hi, :], rb[:, hi, :])
                    while fi < nf:
                        fillers[fi]()
                        fi += 1

                # ---- fused schedule ------------------------------------
                # out_tile(tt) units are deferred toward late slices where
                # attention alone is Act-bound and PE has idle slots; their
                # psum->sbuf copies ride Act there (DVE is busier late).
                out_fill = {1: [0, 1, 2, 3], 2: [4, 5, 6, 7], 3: [8, 9, 10, 11]}
                for u in proj_units(0):
                    u()
                for t in range(NT):
                    fillers = []
                    if t + 1 < NT:  # next slice's projection
                        fillers += list(proj_units(t + 1))
                    fillers += [(lambda tt=tt, a=(t >= 2 and tt % 2 == 1):
                                 out_tile(tt, a))
                                for tt in out_fill.get(t, [])]
                    attention(t, fillers, tail=(t == NT - 1))

    return nc


def _prep_inputs(x, rope_cos, rope_sin, W_attn, b_attn, W_proj, b_proj, bQ, bK):
    """Slice/transpose/pack the full inputs into 8 per-core input maps."""
    assert not np.any(b_attn), "kernel assumes b_attn == 0 (true for this problem)"
    bf = bfloat16
    f = np.float32
    in_maps = []
    # per-batch tensors
    xtb = []
    for b in range(B):
        xT = np.asarray(x[b]).T.astype(bf)  # [C, T]
        xtb.append(np.ascontiguousarray(
            xT.reshape(8, 128, NT, 512).transpose(1, 2, 0, 3)))  # [128,t,c,512]
    cos_r, sin_r = [], []
    for b in range(B):
        ct = np.zeros((128, T), dtype=f)
        st = np.zeros((128, T), dtype=f)
        sT = np.asarray(rope_sin[b]).T  # [RD, T]
        signed = np.concatenate([-sT[0:RD // 2], sT[RD // 2:RD]], axis=0)
        ct[0:RD, :] = np.asarray(rope_cos[b]).T
        ct[64:64 + RD, :] = np.asarray(rope_cos[b]).T
        ct[RD:64, :] = 1.0
        ct[64 + RD:128, :] = 1.0
        st[0:RD, :] = signed
        st[64:64 + RD, :] = signed
        cos_r.append(ct.astype(bf))
        sin_r.append(st.astype(bf))
    tri = np.triu(np.ones((128, 128), dtype=f)).astype(bf)
    pm = np.zeros((128, 128), dtype=f)
    H = RD // 2
    for base in (0, 64):
        for i in range(H):
            pm[base + H + i, base + i] = 1.0      # out[0:16] = in[16:32]
            pm[base + i, base + H + i] = 1.0      # out[16:32] = in[0:16]
    pm = pm.astype(bf)
    W_attn = np.asarray(W_attn)
    W_proj = np.asarray(W_proj)
    bQ = np.asarray(bQ)
    bK = np.asarray(bK)
    for core in range(N_CORES):
        b, g = divmod(core, G)
        qbase = g * 256
        kbase = C + g * 256
        chunk_cols = [
            W_attn[:, qbase:qbase + 128],          # q01
            W_attn[:, kbase:kbase + 128],          # k01
            W_attn[:, qbase + 128:qbase + 256],    # q23
            W_attn[:, kbase + 128:kbase + 256],    # k23
        ]
        w_qk = np.stack([c.astype(bf).reshape(8, 128, 128).transpose(1, 0, 2)
                         for c in chunk_cols], axis=1)  # [128, 4, 8, 128]
        w_v = np.ascontiguousarray(
            W_attn[:, 2 * C + g * 256: 2 * C + (g + 1) * 256]
            .astype(bf).reshape(8, 128, 256).transpose(1, 0, 2))  # [128,8,256]
        w_p = np.ascontiguousarray(
            W_proj[g * 256:(g + 1) * 256, :]
            .astype(bf).reshape(2, 128, C).transpose(1, 0, 2))  # [128,2,1024]
        bias = np.zeros((128, 4), dtype=f)
        # chunk order q01, k01, q23, k23
        for ci, (src, pair) in enumerate(
                [(bQ, 0), (bK, 0), (bQ, 1), (bK, 1)]):
            h0 = g * HPG + pair * 2
            bias[0:64, ci] = src[h0]
            bias[64:128, ci] = src[h0 + 1]
        in_maps.append({
            "x_t": xtb[b],
            "w_qk": np.ascontiguousarray(w_qk),
            "w_v": w_v,
            "w_p": w_p,
            "cos_r": cos_r[b],
            "sin_r": sin_r[b],
            "bias_qk": bias,
            "tri": np.ascontiguousarray(tri),
            "perm": np.ascontiguousarray(pm),
        })
    return in_maps


def _get_nc(loop_k: int = 1):
    key = ("nc", loop_k)
    if key not in _cache:
        _install_waitsplit()
        _cache[key] = _build(loop_k)
    return _cache[key]


def run_spmd(in_maps):
    from concourse.bass_utils import run_bass_kernel_spmd

    nc = _get_nc()
    return run_bass_kernel_spmd(nc, in_maps, core_ids=list(range(N_CORES)))


def kernel(x, rope_cos, rope_sin, W_attn, b_attn, W_proj, b_proj, bQ, bK):
    in_maps = _prep_inputs(x, rope_cos, rope_sin, W_attn, b_attn, W_proj, b_proj,
                           bQ, bK)
    res = run_spmd(in_maps)
    outs = [res.results[c]["out"] for c in range(N_CORES)]
    b_proj = np.asarray(b_proj, dtype=np.float64)
    full = np.empty((B, T, C), dtype=np.float32)
    for b in range(B):
        acc = np.zeros((T, C), dtype=np.float64)
        for g in range(G):
            acc += outs[b * G + g].astype(np.float64)
        full[b] = (acc + b_proj).astype(np.float32)
    return full


# revision 30
# speedup vs baseline: 1.0084x; 1.0084x over previous
"""DisorderedCausalSelfAttention on 8 Trainium2 NeuronCores.

Problem: y = proj(causal_attn(rope_bias(qkv(x)))) with
  B=2, T=2048, C=1024, NH=16, D=64, RD=32 (partial RoPE), per-head
  additive biases bQ/bK applied post-RoPE.

Sharding: core c -> (batch b = c//4, head-group g = c%4 of 4 heads).
Each core computes qkv for its 4 heads, attention, and a partial output
projection (its 256 rows of W_proj); the host sums the 4 partials per
batch and adds b_proj.

Design (vs the phase-serial f32r v1):
  - bf16 storage for every matmul operand (same PE rate as f32r, half
    the DMA traffic and SBUF, no f32r small-moving-dim penalty, DVE
    2x/4x modes); PSUM accumulation stays f32.
  - one fused pipeline over t-slices of 512 positions: qkv projection +
    RoPE for slice t, then causal attention rows qt=t for both head
    pairs, then the output projection rows of slice t with its DMA
    out -- input streaming, compute, and write-back all overlap.
  - next slice's projection and deferred out-projection tiles are
    interleaved as fillers between attention S/exp/AV groups so the PE
    never waits on the Activation engine's exp stream.
  - PSUM (8 banks): one 2-bank FIFO ring (tag "s", bufs=3) carries proj
    pairs, rope-perm pairs, V quads, S tiles and out-proj pairs; the ys
    accumulator (2 banks, bufs=1) is released early by a single wide
    DVE copy to SBUF, with normalization off the critical path; the
    final slice normalizes per column straight from psum so the last
    out-proj tiles start immediately.
  - S matmuls are causally trimmed (columns >= c0 of the q-tile); the
    RoPE bias add is fused into the sin-term add (scalar_tensor_tensor).
  - all output DMAs ride the HWDGE (sync) queue: SWDGE descriptor
    generation costs ~1.3us of Pool-engine time per transfer.
"""

import sys

sys.path.insert(0, "/opt/trn_rl_repo")

import json

import numpy as np
from ml_dtypes import bfloat16

B, T, C, NH, D, RD = 2, 2048, 1024, 16, 64, 32
G = 4  # head-groups (cores per batch)
HPG = NH // G  # heads per group = 4
N_CORES = 8
SCALE = float(D) ** -0.5
NT = T // 512  # 4 t-slices
NK = T // 128  # 16 k tiles

_cache = {}


# ---------------------------------------------------------------------------
# Workaround: this container's walrus build accepts at most ONE sync-wait
# command on most instructions, while Tile emits up to ~4.  Split excess
# waits into EventSemaphore instructions inserted immediately before, on the
# same engine (same-queue program order keeps semantics).
# ---------------------------------------------------------------------------
def _split_waits(bj: bytes, es_cap: int = 2) -> bytes:
    d = json.loads(bj)
    for fn in d.get("functions", []):
        for bb in fn.get("blocks", []):
            new = []
            for inst in bb.get("instructions", []):
                si = inst.get("sync_info") or {}
                w = si.get("on_wait") or []
                lim = es_cap if inst.get("opcode") == "EventSemaphore" else 1
                if len(w) > lim:
                    keep = w[-lim:]
                    mv = w[:-lim]
                    for ci in range(0, len(mv), es_cap):
                        new.append({
                            "debug": inst.get("debug"),
                            "engine": inst["engine"],
                            "ins": [], "outs": [],
                            "name": f"{inst['name']}_ws{ci}",
                            "opcode": "EventSemaphore",
                            "sync_info": {"on_update": [],
                                          "on_wait": mv[ci:ci + es_cap]},
                        })
                    si["on_wait"] = keep
                new.append(inst)
            bb["instructions"] = new
    return json.dumps(d).encode()


def _install_waitsplit():
    from concourse import bass2jax, bass_utils

    if getattr(bass2jax.compile_bir_kernel, "_waitsplit", False):
        return
    orig = bass_utils.compile_bir_kernel

    def patched(bj, tmpdir, neff_name="file.neff"):
        return orig(_split_waits(bj), tmpdir, neff_name)

    patched._waitsplit = True
    bass2jax.compile_bir_kernel = patched


# ---------------------------------------------------------------------------
# Kernel builder (one SPMD program; per-core data differs via in_maps)
# ---------------------------------------------------------------------------
def _build(loop_k: int = 1):
    import concourse.bass as bass
    import concourse.tile as tile
    from concourse import mybir

    f32 = mybir.dt.float32
    bf16 = mybir.dt.bfloat16
    Exp = mybir.ActivationFunctionType.Exp

    nc = bass.Bass("TRN2")

    # DRAM parameters, host-packed to match SBUF layouts exactly.
    xt = nc.declare_dram_parameter("x_t", [128, NT, 8, 512], bf16, isOutput=False)
    wqk = nc.declare_dram_parameter("w_qk", [128, 4, 8, 128], bf16, isOutput=False)
    wv = nc.declare_dram_parameter("w_v", [128, 8, 256], bf16, isOutput=False)
    wp = nc.declare_dram_parameter("w_p", [128, 2, 1024], bf16, isOutput=False)
    cosr = nc.declare_dram_parameter("cos_r", [128, T], bf16, isOutput=False)
    sinr = nc.declare_dram_parameter("sin_r", [128, T], bf16, isOutput=False)
    bqk = nc.declare_dram_parameter("bias_qk", [128, 4], f32, isOutput=False)
    trim = nc.declare_dram_parameter("tri", [128, 128], bf16, isOutput=False)
    perm = nc.declare_dram_parameter("perm", [128, 128], bf16, isOutput=False)
    out = nc.declare_dram_parameter("out", [T, C], bf16, isOutput=True)

    with tile.TileContext(nc) as tc:
      for _rep in range(loop_k):
        with tc.tile_pool(name="persist", bufs=1) as pp:
            XT = pp.tile([128, NT, 8, 512], bf16)
            WQK = pp.tile([128, 4, 8, 128], bf16)   # chunk order q01,k01,q23,k23
            WV = pp.tile([128, 8, 256], bf16)
            WP = pp.tile([128, 2, 1024], bf16)
            COS = pp.tile([128, T], bf16)
            SIN = pp.tile([128, T], bf16)
            BQK = pp.tile([128, 4], f32)
            TRI = pp.tile([128, 128], bf16)
            PERM = pp.tile([128, 128], bf16)
            QK = pp.tile([128, 4, T], bf16)          # chunks q01,k01,q23,k23
            V4 = pp.tile([128, NK, HPG, 2 * D], bf16)
            YT = pp.tile([128, 2, T], bf16)

            # weights on the SWDGE (gpsimd) queue in consumption order;
            # first chunk split small so the first matmul can start early.
            nc.sync.dma_start(out=WQK[:, 0:1], in_=wqk[:, 0:1])
            nc.gpsimd.dma_start(out=WQK[:, 1:2], in_=wqk[:, 1:2])
            nc.gpsimd.dma_start(out=WQK[:, 2:4], in_=wqk[:, 2:4])
            nc.gpsimd.dma_start(out=WV, in_=wv[:, :, :])
            nc.gpsimd.dma_start(out=WP, in_=wp[:, :, :])
            nc.gpsimd.memset(V4[:, :, :, D:], 1.0)

            def _rope_slices(t):
                sl = slice(t * 512, (t + 1) * 512)
                nc.sync.dma_start(out=COS[:, sl], in_=cosr[:, sl])
                nc.sync.dma_start(out=SIN[:, sl], in_=sinr[:, sl])

            nc.sync.dma_start(out=XT[:, 0, 0:2], in_=xt[:, 0, 0:2])
            nc.sync.dma_start(out=XT[:, 0, 2:8], in_=xt[:, 0, 2:8])
            _rope_slices(0)
            nc.sync.dma_start(out=BQK, in_=bqk[:, :])
            nc.sync.dma_start(out=TRI, in_=trim[:, :])
            nc.sync.dma_start(out=PERM, in_=perm[:, :])
            for t in range(1, NT):
                nc.sync.dma_start(out=XT[:, t], in_=xt[:, t])
                _rope_slices(t)

            with (
                tc.tile_pool(name="ps", bufs=2, space="PSUM") as ps,
                tc.tile_pool(name="sb", bufs=3) as sb,
            ):
                # ---- emission units ------------------------------------
                def proj_pair(t, mp, j, st):
                    """half of a qk-projection chunk pair for slice t."""
                    tc0, tc1 = t * 512, (t + 1) * 512
                    if j == 0:
                        st["pa"] = ps.tile([128, 2, 512], f32, tag="s", bufs=3,
                                           name=f"pa_{t}_{mp}")
                    pa = st["pa"]
                    for c in range(8):
                        nc.tensor.matmul(
                            pa[:, j, :],
                            WQK[:, 2 * mp + j, c, :],
                            XT[:, t, c, :],
                            start=(c == 0), stop=(c == 7),
                        )
                    if j == 1:
                        nc.vector.tensor_copy(
                            QK[:, 2 * mp:2 * mp + 2, tc0:tc1], pa)

                def rope_pair(t, mp):
                    """RoPE for chunk pair mp of slice t (after proj_pair)."""
                    tc0, tc1 = t * 512, (t + 1) * 512
                    pr = ps.tile([128, 2, 512], f32, tag="s", bufs=3,
                                 name=f"pr_{t}_{mp}")
                    for j in range(2):
                        nc.tensor.matmul(
                            pr[:, j, :], PERM,
                            QK[:, 2 * mp + j, tc0:tc1],
                            start=True, stop=True)
                    tmp = sb.tile([128, 2, 512], bf16, tag="tmp",
                                  name=f"tmp_{t}_{mp}")
                    nc.vector.tensor_mul(
                        tmp, pr,
                        SIN[:, None, tc0:tc1].broadcast_to([128, 2, 512]))
                    qsl = QK[:, 2 * mp:2 * mp + 2, tc0:tc1]
                    nc.vector.tensor_mul(
                        qsl, qsl,
                        COS[:, None, tc0:tc1].broadcast_to([128, 2, 512]))
                    for j in range(2):
                        m = 2 * mp + j
                        nc.vector.scalar_tensor_tensor(
                            QK[:, m, tc0:tc1], tmp[:, j, :], BQK[:, m:m + 1],
                            QK[:, m, tc0:tc1],
                            mybir.AluOpType.add, mybir.AluOpType.add)

                def v_pair(t, kq, j, st):
                    """half of a V-projection k-tile pair of slice t."""
                    if j == 0:
                        st["pv"] = ps.tile([128, 2, 512], f32, tag="s", bufs=3,
                                           name=f"pv_{t}_{kq}")
                    pv = st["pv"]
                    for c in range(8):
                        nc.tensor.matmul(
                            pv[:, j, 0:256],
                            XT[:, t, c, (2 * kq + j) * 128:
                               (2 * kq + j) * 128 + 128],
                            WV[:, c, :],
                            start=(c == 0), stop=(c == 7),
                        )
                    if j == 1:
                        nc.scalar.copy(
                            V4[:, 4 * t + 2 * kq:4 * t + 2 * kq + 2, :, 0:D],
                            pv[:, :, 0:256].rearrange(
                                "p j (h d) -> p j h d", h=HPG),
                        )

                def out_tile(tt, on_act=False):
                    """output projection for row-tile tt (128 rows)."""
                    po = ps.tile([128, 2, 512], f32, tag="s", bufs=3,
                                 name=f"po_{tt}")
                    for n in range(2):
                        for cc in range(2):
                            nc.tensor.matmul(
                                po[:, n, :],
                                YT[:, cc, tt * 128:(tt + 1) * 128],
                                WP[:, cc, n * 512:(n + 1) * 512],
                                start=(cc == 0), stop=(cc == 1),
                            )
                    ob = sb.tile([128, 1024], bf16, tag="ob",
                                 name=f"ob_{tt}")
                    obv = ob.rearrange("p (n w) -> p n w", n=2)
                    if on_act:
                        nc.scalar.copy(obv, po)
                    else:
                        nc.vector.tensor_copy(obv, po)
                    nc.sync.dma_start(out=out[tt * 128:(tt + 1) * 128, :],
                                      in_=ob)

                def proj_units(t):
                    stA, stB, svA, svB = {}, {}, {}, {}
                    yield lambda: proj_pair(t, 0, 0, stA)
                    yield lambda: proj_pair(t, 0, 1, stA)
                    yield lambda: rope_pair(t, 0)
                    yield lambda: v_pair(t, 0, 0, svA)
                    yield lambda: v_pair(t, 0, 1, svA)
                    yield lambda: proj_pair(t, 1, 0, stB)
                    yield lambda: proj_pair(t, 1, 1, stB)
                    yield lambda: rope_pair(t, 1)
                    yield lambda: v_pair(t, 1, 0, svB)
                    yield lambda: v_pair(t, 1, 1, svB)

                def attention(t, fillers, tail=False):
                    """Causal attention rows qt=t for both head pairs,
                    interleaving filler units (next slice's projection /
                    previous slice's output projection) into the PE stream
                    to hide exp/rope latency."""
                    tc0, tc1 = t * 512, (t + 1) * 512
                    nkt = 4 * t + 4
                    ngroups = 2 * nkt
                    nf = len(fillers)
                    fi = 0
                    gi = 0
                    for hp in range(2):
                        qc, kc = 2 * hp, 2 * hp + 1
                        ys = ps.tile([128, 2, 512], f32, tag="ys", bufs=1,
                                     name=f"ys_{t}_{hp}")
                        for kt in range(nkt):
                            j = kt - 4 * t
                            c0 = max(j, 0) * 128
                            s = ps.tile([128, 2, 512], f32, tag="s", bufs=3,
                                        name=f"s_{t}_{hp}_{kt}")
                            for hi in range(2):
                                o = hi * 64
                                nc.tensor.matmul(
                                    s[:, hi, c0:],
                                    QK[o:o + 64, kc, kt * 128:(kt + 1) * 128],
                                    QK[o:o + 64, qc, tc0 + c0:tc1],
                                    start=True, stop=True,
                                )
                            p = sb.tile([128, 2, 512], bf16, tag="p",
                                        bufs=4, name=f"p_{t}_{hp}_{kt}")
                            nc.scalar.activation(p[:, :, c0:], s[:, :, c0:],
                                                 Exp, scale=SCALE)
                            if j >= 0:
                                nc.vector.tensor_mul(
                                    p[:, :, c0:c0 + 128], p[:, :, c0:c0 + 128],
                                    TRI[:, None, :].broadcast_to([128, 2, 128]))
                            # filler between S/exp and AV hides exp latency
                            gi += 1
                            while fi < nf and fi + 1 <= (gi * nf) // ngroups:
                                fillers[fi]()
                                fi += 1
                            for hi in range(2):
                                nc.tensor.matmul(
                                    ys[:, hi, c0:],
                                    V4[:, kt, 2 * hp + hi, :],
                                    p[:, hi, c0:],
                                    start=(kt == 0), stop=(kt == nkt - 1),
                                )
                        # release ys with one wide copy to SBUF, then
                        # normalize off the critical path: rows 64:128 hold
                        # the softmax denominators (ones-block matmul).
                        if tail and hp == 1:
                            # last stretch: normalize straight from ys psum,
                            # per column, so each final out-proj tile starts
                            # as early as possible (nothing reuses ys after).
                            rb = sb.tile([64, 2, 512], f32, tag="rb",
                                         name=f"rb_{t}_{hp}")
                            for q in range(4):
                                ql, qh = q * 128, (q + 1) * 128
                                nc.vector.reciprocal(
                                    rb[:, :, ql:qh], ys[64:128, :, ql:qh])
                                for hi in range(2):
                                    o = hi * 64
                                    nc.vector.tensor_mul(
                                        YT[o:o + 64, hp, tc0 + ql:tc0 + qh],
                                        ys[0:D, hi, ql:qh],
                                        rb[:, hi, ql:qh])
                                out_tile(4 * t + q, q % 2 == 0)
                        else:
                            ysb = sb.tile([128, 2, 512], f32, tag="ysb",
                                          bufs=2, name=f"ysb_{t}_{hp}")
                            nc.vector.tensor_copy(ysb, ys)
                            rb = sb.tile([64, 2, 512], f32, tag="rb",
                                         name=f"rb_{t}_{hp}")
                            nc.vector.reciprocal(rb, ysb[64:128, :, :])
                            for hi in range(2):
                                o = hi * 64
                                nc.vector.tensor_mul(
                                    YT[o:o + 64, hp, tc0:tc1],
                                    ysb[0:D, hi, :], rb[:, hi, :])
                    while fi < nf:
                        fillers[fi]()
                        fi += 1

                # ---- fused schedule ------------------------------------
                # out_tile(tt) units are deferred toward late slices where
                # attention alone is Act-bound and PE has idle slots; their
                # psum->sbuf copies ride Act there (DVE is busier late).
                out_fill = {1: [0, 1, 2, 3], 2: [4, 5, 6, 7], 3: [8, 9, 10, 11]}
                for u in proj_units(0):
                    u()
                for t in range(NT):
                    fillers = []
                    if t + 1 < NT:  # next slice's projection
                        fillers += list(proj_units(t + 1))
                    fillers += [(lambda tt=tt, a=(t >= 2 and tt % 2 == 1):
                                 out_tile(tt, a))
                                for tt in out_fill.get(t, [])]
                    attention(t, fillers, tail=(t == NT - 1))

    return nc


def _prep_inputs(x, rope_cos, rope_sin, W_attn, b_attn, W_proj, b_proj, bQ, bK):
    """Slice/transpose/pack the full inputs into 8 per-core input maps."""
    assert not np.any(b_attn), "kernel assumes b_attn == 0 (true for this problem)"
    bf = bfloat16
    f = np.float32
    in_maps = []
    # per-batch tensors
    xtb = []
    for b in range(B):
        xT = np.asarray(x[b]).T.astype(bf)  # [C, T]
        xtb.append(np.ascontiguousarray(
            xT.reshape(8, 128, NT, 512).transpose(1, 2, 0, 3)))  # [128,t,c,512]
    cos_r, sin_r = [], []
    for b in range(B):
        ct = np.zeros((128, T), dtype=f)
        st = np.zeros((128, T), dtype=f)
        sT = np.asarray(rope_sin[b]).T  # [RD, T]
        signed = np.concatenate([-sT[0:RD // 2], sT[RD // 2:RD]], axis=0)
        ct[0:RD, :] = np.asarray(rope_cos[b]).T
        ct[64:64 + RD, :] = np.asarray(rope_cos[b]).T
        ct[RD:64, :] = 1.0
        ct[64 + RD:128, :] = 1.0
        st[0:RD, :] = signed
        st[64:64 + RD, :] = signed
        cos_r.append(ct.astype(bf))
        sin_r.append(st.astype(bf))
    tri = np.triu(np.ones((128, 128), dtype=f)).astype(bf)
    pm = np.zeros((128, 128), dtype=f)
    H = RD // 2
    for base in (0, 64):
        for i in range(H):
            pm[base + H + i, base + i] = 1.0      # out[0:16] = in[16:32]
            pm[base + i, base + H + i] = 1.0      # out[16:32] = in[0:16]
    pm = pm.astype(bf)
    W_attn = np.asarray(W_attn)
    W_proj = np.asarray(W_proj)
    bQ = np.asarray(bQ)
    bK = np.asarray(bK)
    for core in range(N_CORES):
        b, g = divmod(core, G)
        qbase = g * 256
        kbase = C + g * 256
        chunk_cols = [
            W_attn[:, qbase:qbase + 128],          # q01
            W_attn[:, kbase:kbase + 128],          # k01
            W_attn[:, qbase + 128:qbase + 256],    # q23
            W_attn[:, kbase + 128:kbase + 256],    # k23
        ]
        w_qk = np.stack([c.astype(bf).reshape(8, 128, 128).transpose(1, 0, 2)
                         for c in chunk_cols], axis=1)  # [128, 4, 8, 128]
        w_v = np.ascontiguousarray(
            W_attn[:, 2 * C + g * 256: 2 * C + (g + 1) * 256]
            .astype(bf).reshape(8, 128, 256).transpose(1, 0, 2))  # [128,8,256]
        w_p = np.ascontiguousarray(
            W_proj[g * 256:(g + 1) * 256, :]
            .astype(bf).reshape(2, 128, C).transpose(1, 0, 2))  # [128,2,1024]
        bias = np.zeros((128, 4), dtype=f)
        # chunk order q01, k01, q23, k23
        for ci, (src, pair) in enumerate(
                [(bQ, 0), (bK, 0), (bQ, 1), (bK, 1)]):
            h0 = g * HPG + pair * 2
            bias[0:64, ci] = src[h0]
            bias[64:128, ci] = src[h0 + 1]
        in_maps.append({
            "x_t": xtb[b],
            "w_qk": np.ascontiguousarray(w_qk),
            "w_v": w_v,
            "w_p": w_p,
            "cos_r": cos_r[b],
            "sin_r": sin_r[b],
            "bias_qk": bias,
            "tri": np.ascontiguousarray(tri),
            "perm": np.ascontiguousarray(pm),
        })
    return in_maps


def _get_nc(loop_k: int = 1):
    key = ("nc", loop_k)
    if key not in _cache:
        _install_waitsplit()
        _cache[key] = _build(loop_k)
    return _cache[key]


def run_spmd(in_maps):
    from concourse.bass_utils import run_bass_kernel_spmd

    nc = _get_nc()
    return run_bass_kernel_spmd(nc, in_maps, core_ids=list(range(N_CORES)))


def kernel(x, rope_cos, rope_sin, W_attn, b_attn, W_proj, b_proj, bQ, bK):
    in_maps = _prep_inputs(x, rope_cos, rope_sin, W_attn, b_attn, W_proj, b_proj,
                           bQ, bK)
    res = run_spmd(in_maps)
    outs = [res.results[c]["out"] for c in range(N_CORES)]
    b_proj = np.asarray(b_proj, dtype=np.float64)
    full = np.empty((B, T, C), dtype=np.float32)
    for b in range(B):
        acc = np.zeros((T, C), dtype=np.float64)
        for g in range(G):
            acc += outs[b * G + g].astype(np.float64)
        full[b] = (acc + b_proj).astype(np.float32)
    return full


# revision 31
# speedup vs baseline: 1.0461x; 1.0374x over previous
"""DisorderedCausalSelfAttention on 8 Trainium2 NeuronCores.

Problem: y = proj(causal_attn(rope_bias(qkv(x)))) with
  B=2, T=2048, C=1024, NH=16, D=64, RD=32 (partial RoPE), per-head
  additive biases bQ/bK applied post-RoPE.

Sharding: core c -> (batch b = c//4, head-group g = c%4 of 4 heads).
Each core computes qkv for its 4 heads, attention, and a partial output
projection (its 256 rows of W_proj); the host sums the 4 partials per
batch and adds b_proj.

Design (vs the phase-serial f32r v1):
  - bf16 storage for every matmul operand (same PE rate as f32r, half
    the DMA traffic and SBUF, no f32r small-moving-dim penalty, DVE
    2x/4x modes); PSUM accumulation stays f32.
  - one fused pipeline over t-slices of 512 positions: qkv projection +
    RoPE for slice t, then causal attention rows qt=t for both head
    pairs, then the output projection rows of slice t with its DMA
    out -- input streaming, compute, and write-back all overlap.
  - next slice's projection and deferred out-projection tiles are
    interleaved as fillers between attention S/exp/AV groups so the PE
    never waits on the Activation engine's exp stream.
  - PSUM (8 banks): one 2-bank FIFO ring (tag "s", bufs=3) carries proj
    pairs, rope-perm pairs, V quads, S tiles and out-proj pairs; the ys
    accumulator (2 banks, bufs=1) is released early by a single wide
    DVE copy to SBUF, with normalization off the critical path; the
    final slice normalizes per column straight from psum so the last
    out-proj tiles start immediately.
  - S matmuls are causally trimmed (columns >= c0 of the q-tile); the
    RoPE bias add is fused into the sin-term add (scalar_tensor_tensor).
  - all output DMAs ride the HWDGE (sync) queue: SWDGE descriptor
    generation costs ~1.3us of Pool-engine time per transfer.
"""

import sys

sys.path.insert(0, "/opt/trn_rl_repo")

import json

import numpy as np
from ml_dtypes import bfloat16

B, T, C, NH, D, RD = 2, 2048, 1024, 16, 64, 32
G = 4  # head-groups (cores per batch)
HPG = NH // G  # heads per group = 4
N_CORES = 8
SCALE = float(D) ** -0.5
NT = T // 512  # 4 t-slices
NK = T // 128  # 16 k tiles

_cache = {}


# ---------------------------------------------------------------------------
# Workaround: this container's walrus build accepts at most ONE sync-wait
# command on most instructions, while Tile emits up to ~4.  Split excess
# waits into EventSemaphore instructions inserted immediately before, on the
# same engine (same-queue program order keeps semantics).
# ---------------------------------------------------------------------------
def _split_waits(bj: bytes, es_cap: int = 2) -> bytes:
    d = json.loads(bj)
    for fn in d.get("functions", []):
        for bb in fn.get("blocks", []):
            new = []
            for inst in bb.get("instructions", []):
                si = inst.get("sync_info") or {}
                w = si.get("on_wait") or []
                lim = es_cap if inst.get("opcode") == "EventSemaphore" else 1
                if len(w) > lim:
                    keep = w[-lim:]
                    mv = w[:-lim]
                    for ci in range(0, len(mv), es_cap):
                        new.append({
                            "debug": inst.get("debug"),
                            "engine": inst["engine"],
                            "ins": [], "outs": [],
                            "name": f"{inst['name']}_ws{ci}",
                            "opcode": "EventSemaphore",
                            "sync_info": {"on_update": [],
                                          "on_wait": mv[ci:ci + es_cap]},
                        })
                    si["on_wait"] = keep
                new.append(inst)
            bb["instructions"] = new
    return json.dumps(d).encode()


def _install_waitsplit():
    from concourse import bass2jax, bass_utils

    if getattr(bass2jax.compile_bir_kernel, "_waitsplit", False):
        return
    orig = bass_utils.compile_bir_kernel

    def patched(bj, tmpdir, neff_name="file.neff"):
        return orig(_split_waits(bj), tmpdir, neff_name)

    patched._waitsplit = True
    bass2jax.compile_bir_kernel = patched


# ---------------------------------------------------------------------------
# Kernel builder (one SPMD program; per-core data differs via in_maps)
# ---------------------------------------------------------------------------
def _build(loop_k: int = 1):
    import concourse.bass as bass
    import concourse.tile as tile
    from concourse import mybir

    f32 = mybir.dt.float32
    bf16 = mybir.dt.bfloat16
    Exp = mybir.ActivationFunctionType.Exp

    nc = bass.Bass("TRN2")

    # DRAM parameters, host-packed to match SBUF layouts exactly.
    xt = nc.declare_dram_parameter("x_t", [128, NT, 8, 512], bf16, isOutput=False)
    wqk = nc.declare_dram_parameter("w_qk", [128, 4, 8, 128], bf16, isOutput=False)
    wv = nc.declare_dram_parameter("w_v", [128, 8, 256], bf16, isOutput=False)
    wp = nc.declare_dram_parameter("w_p", [128, 2, 1024], bf16, isOutput=False)
    cosr = nc.declare_dram_parameter("cos_r", [128, T], bf16, isOutput=False)
    sinr = nc.declare_dram_parameter("sin_r", [128, T], bf16, isOutput=False)
    bqk = nc.declare_dram_parameter("bias_qk", [128, 4], f32, isOutput=False)
    trim = nc.declare_dram_parameter("tri", [128, 128], bf16, isOutput=False)
    out = nc.declare_dram_parameter("out", [T, C], bf16, isOutput=True)

    with tile.TileContext(nc) as tc:
      for _rep in range(loop_k):
        with tc.tile_pool(name="persist", bufs=1) as pp:
            XT = pp.tile([128, NT, 8, 512], bf16)
            WQK = pp.tile([128, 4, 8, 128], bf16)   # chunk order q01,k01,q23,k23
            WV = pp.tile([128, 8, 256], bf16)
            WP = pp.tile([128, 2, 1024], bf16)
            COS = pp.tile([128, T], bf16)
            SIN = pp.tile([128, T], bf16)
            BQK = pp.tile([128, 4], f32)
            TRI = pp.tile([128, 128], bf16)
            QK = pp.tile([128, 4, T], bf16)          # chunks q01,k01,q23,k23
            V4 = pp.tile([128, NK, HPG, 2 * D], bf16)
            YT = pp.tile([128, 2, T], bf16)

            # weights on the SWDGE (gpsimd) queue in consumption order;
            # first chunk split small so the first matmul can start early.
            nc.sync.dma_start(out=WQK[:, 0:1], in_=wqk[:, 0:1])
            nc.gpsimd.dma_start(out=WQK[:, 1:2], in_=wqk[:, 1:2])
            nc.gpsimd.dma_start(out=WQK[:, 2:4], in_=wqk[:, 2:4])
            nc.gpsimd.dma_start(out=WV, in_=wv[:, :, :])
            nc.gpsimd.dma_start(out=WP, in_=wp[:, :, :])
            nc.gpsimd.memset(V4[:, :, :, D:], 1.0)

            def _rope_slices(t):
                sl = slice(t * 512, (t + 1) * 512)
                nc.sync.dma_start(out=COS[:, sl], in_=cosr[:, sl])
                nc.sync.dma_start(out=SIN[:, sl], in_=sinr[:, sl])

            nc.sync.dma_start(out=XT[:, 0, 0:2], in_=xt[:, 0, 0:2])
            nc.sync.dma_start(out=XT[:, 0, 2:8], in_=xt[:, 0, 2:8])
            _rope_slices(0)
            nc.sync.dma_start(out=BQK, in_=bqk[:, :])
            nc.sync.dma_start(out=TRI, in_=trim[:, :])
            for t in range(1, NT):
                nc.sync.dma_start(out=XT[:, t], in_=xt[:, t])
                _rope_slices(t)

            with (
                tc.tile_pool(name="ps", bufs=2, space="PSUM") as ps,
                tc.tile_pool(name="sb", bufs=3) as sb,
            ):
                # ---- emission units ------------------------------------
                def proj_pair(t, mp, j, st):
                    """half of a qk-projection chunk pair for slice t."""
                    tc0, tc1 = t * 512, (t + 1) * 512
                    if j == 0:
                        st["pa"] = ps.tile([128, 2, 512], f32, tag="s", bufs=3,
                                           name=f"pa_{t}_{mp}")
                    pa = st["pa"]
                    for c in range(8):
                        nc.tensor.matmul(
                            pa[:, j, :],
                            WQK[:, 2 * mp + j, c, :],
                            XT[:, t, c, :],
                            start=(c == 0), stop=(c == 7),
                        )
                    if j == 1:
                        nc.vector.tensor_copy(
                            QK[:, 2 * mp:2 * mp + 2, tc0:tc1], pa)

                # rotate-half partition swap within each 32-quadrant; the
                # pass-dim quadrants get swapped garbage that SIN's zero
                # rows cancel.
                SWAP_MASK = [(i + 16) % 32 for i in range(32)]

                def rope_pair(t, mp):
                    """RoPE for chunk pair mp of slice t (after proj_pair)."""
                    tc0, tc1 = t * 512, (t + 1) * 512
                    qsl = QK[:, 2 * mp:2 * mp + 2, tc0:tc1]
                    tmp = sb.tile([128, 2, 512], bf16, tag="tmp",
                                  name=f"tmp_{t}_{mp}")
                    nc.vector.stream_shuffle(tmp, qsl, SWAP_MASK)
                    nc.vector.tensor_mul(
                        tmp, tmp,
                        SIN[:, None, tc0:tc1].broadcast_to([128, 2, 512]))
                    nc.vector.tensor_mul(
                        qsl, qsl,
                        COS[:, None, tc0:tc1].broadcast_to([128, 2, 512]))
                    for j in range(2):
                        m = 2 * mp + j
                        nc.vector.scalar_tensor_tensor(
                            QK[:, m, tc0:tc1], tmp[:, j, :], BQK[:, m:m + 1],
                            QK[:, m, tc0:tc1],
                            mybir.AluOpType.add, mybir.AluOpType.add)

                def v_pair(t, kq, j, st):
                    """half of a V-projection k-tile pair of slice t."""
                    if j == 0:
                        st["pv"] = ps.tile([128, 2, 512], f32, tag="s", bufs=3,
                                           name=f"pv_{t}_{kq}")
                    pv = st["pv"]
                    for c in range(8):
                        nc.tensor.matmul(
                            pv[:, j, 0:256],
                            XT[:, t, c, (2 * kq + j) * 128:
                               (2 * kq + j) * 128 + 128],
                            WV[:, c, :],
                            start=(c == 0), stop=(c == 7),
                        )
                    if j == 1:
                        nc.scalar.copy(
                            V4[:, 4 * t + 2 * kq:4 * t + 2 * kq + 2, :, 0:D],
                            pv[:, :, 0:256].rearrange(
                                "p j (h d) -> p j h d", h=HPG),
                        )

                def out_tile(tt, on_act=False):
                    """output projection for row-tile tt (128 rows)."""
                    po = ps.tile([128, 2, 512], f32, tag="s", bufs=3,
                                 name=f"po_{tt}")
                    for n in range(2):
                        for cc in range(2):
                            nc.tensor.matmul(
                                po[:, n, :],
                                YT[:, cc, tt * 128:(tt + 1) * 128],
                                WP[:, cc, n * 512:(n + 1) * 512],
                                start=(cc == 0), stop=(cc == 1),
                            )
                    ob = sb.tile([128, 1024], bf16, tag="ob",
                                 name=f"ob_{tt}")
                    obv = ob.rearrange("p (n w) -> p n w", n=2)
                    if on_act:
                        nc.scalar.copy(obv, po)
                    else:
                        nc.vector.tensor_copy(obv, po)
                    nc.sync.dma_start(out=out[tt * 128:(tt + 1) * 128, :],
                                      in_=ob)

                def proj_units(t):
                    stA, stB, svA, svB = {}, {}, {}, {}
                    yield lambda: proj_pair(t, 0, 0, stA)
                    yield lambda: proj_pair(t, 0, 1, stA)
                    yield lambda: rope_pair(t, 0)
                    yield lambda: v_pair(t, 0, 0, svA)
                    yield lambda: v_pair(t, 0, 1, svA)
                    yield lambda: proj_pair(t, 1, 0, stB)
                    yield lambda: proj_pair(t, 1, 1, stB)
                    yield lambda: rope_pair(t, 1)
                    yield lambda: v_pair(t, 1, 0, svB)
                    yield lambda: v_pair(t, 1, 1, svB)

                def attention(t, fillers, tail=False):
                    """Causal attention rows qt=t for both head pairs,
                    interleaving filler units (next slice's projection /
                    previous slice's output projection) into the PE stream
                    to hide exp/rope latency."""
                    tc0, tc1 = t * 512, (t + 1) * 512
                    nkt = 4 * t + 4
                    ngroups = 2 * nkt
                    nf = len(fillers)
                    fi = 0
                    gi = 0
                    for hp in range(2):
                        qc, kc = 2 * hp, 2 * hp + 1
                        ys = ps.tile([128, 2, 512], f32, tag="ys", bufs=1,
                                     name=f"ys_{t}_{hp}")
                        for kt in range(nkt):
                            j = kt - 4 * t
                            c0 = max(j, 0) * 128
                            s = ps.tile([128, 2, 512], f32, tag="s", bufs=3,
                                        name=f"s_{t}_{hp}_{kt}")
                            for hi in range(2):
                                o = hi * 64
                                nc.tensor.matmul(
                                    s[:, hi, c0:],
                                    QK[o:o + 64, kc, kt * 128:(kt + 1) * 128],
                                    QK[o:o + 64, qc, tc0 + c0:tc1],
                                    start=True, stop=True,
                                )
                            p = sb.tile([128, 2, 512], bf16, tag="p",
                                        bufs=4, name=f"p_{t}_{hp}_{kt}")
                            nc.scalar.activation(p[:, :, c0:], s[:, :, c0:],
                                                 Exp, scale=SCALE)
                            if j >= 0:
                                nc.vector.tensor_mul(
                                    p[:, :, c0:c0 + 128], p[:, :, c0:c0 + 128],
                                    TRI[:, None, :].broadcast_to([128, 2, 128]))
                            # filler between S/exp and AV hides exp latency
                            gi += 1
                            while fi < nf and fi + 1 <= (gi * nf) // ngroups:
                                fillers[fi]()
                                fi += 1
                            for hi in range(2):
                                nc.tensor.matmul(
                                    ys[:, hi, c0:],
                                    V4[:, kt, 2 * hp + hi, :],
                                    p[:, hi, c0:],
                                    start=(kt == 0), stop=(kt == nkt - 1),
                                )
                        # release ys with one wide copy to SBUF, then
                        # normalize off the critical path: rows 64:128 hold
                        # the softmax denominators (ones-block matmul).
                        if tail and hp == 1:
                            # last stretch: normalize straight from ys psum,
                            # per column, so each final out-proj tile starts
                            # as early as possible (nothing reuses ys after).
                            rb = sb.tile([64, 2, 512], f32, tag="rb",
                                         name=f"rb_{t}_{hp}")
                            for q in range(4):
                                ql, qh = q * 128, (q + 1) * 128
                                nc.vector.reciprocal(
                                    rb[:, :, ql:qh], ys[64:128, :, ql:qh])
                                for hi in range(2):
                                    o = hi * 64
                                    nc.vector.tensor_mul(
                                        YT[o:o + 64, hp, tc0 + ql:tc0 + qh],
                                        ys[0:D, hi, ql:qh],
                                        rb[:, hi, ql:qh])
                                out_tile(4 * t + q, q % 2 == 0)
                        else:
                            ysb = sb.tile([128, 2, 512], f32, tag="ysb",
                                          bufs=2, name=f"ysb_{t}_{hp}")
                            nc.vector.tensor_copy(ysb, ys)
                            rb = sb.tile([64, 2, 512], f32, tag="rb",
                                         name=f"rb_{t}_{hp}")
                            nc.vector.reciprocal(rb, ysb[64:128, :, :])
                            for hi in range(2):
                                o = hi * 64
                                nc.vector.tensor_mul(
                                    YT[o:o + 64, hp, tc0:tc1],
                                    ysb[0:D, hi, :], rb[:, hi, :])
                    while fi < nf:
                        fillers[fi]()
                        fi += 1

                # ---- fused schedule ------------------------------------
                # out_tile(tt) units are deferred toward late slices where
                # attention alone is Act-bound and PE has idle slots; their
                # psum->sbuf copies ride Act there (DVE is busier late).
                out_fill = {1: [0, 1, 2, 3], 2: [4, 5, 6, 7], 3: [8, 9, 10, 11]}
                for u in proj_units(0):
                    u()
                for t in range(NT):
                    fillers = []
                    if t + 1 < NT:  # next slice's projection
                        fillers += list(proj_units(t + 1))
                    fillers += [(lambda tt=tt, a=(t >= 2 and tt % 2 == 1):
                                 out_tile(tt, a))
                                for tt in out_fill.get(t, [])]
                    attention(t, fillers, tail=(t == NT - 1))

    return nc


def _prep_inputs(x, rope_cos, rope_sin, W_attn, b_attn, W_proj, b_proj, bQ, bK):
    """Slice/transpose/pack the full inputs into 8 per-core input maps."""
    assert not np.any(b_attn), "kernel assumes b_attn == 0 (true for this problem)"
    bf = bfloat16
    f = np.float32
    in_maps = []
    # per-batch tensors
    xtb = []
    for b in range(B):
        xT = np.asarray(x[b]).T.astype(bf)  # [C, T]
        xtb.append(np.ascontiguousarray(
            xT.reshape(8, 128, NT, 512).transpose(1, 2, 0, 3)))  # [128,t,c,512]
    cos_r, sin_r = [], []
    for b in range(B):
        ct = np.zeros((128, T), dtype=f)
        st = np.zeros((128, T), dtype=f)
        sT = np.asarray(rope_sin[b]).T  # [RD, T]
        signed = np.concatenate([-sT[0:RD // 2], sT[RD // 2:RD]], axis=0)
        ct[0:RD, :] = np.asarray(rope_cos[b]).T
        ct[64:64 + RD, :] = np.asarray(rope_cos[b]).T
        ct[RD:64, :] = 1.0
        ct[64 + RD:128, :] = 1.0
        st[0:RD, :] = signed
        st[64:64 + RD, :] = signed
        cos_r.append(ct.astype(bf))
        sin_r.append(st.astype(bf))
    tri = np.triu(np.ones((128, 128), dtype=f)).astype(bf)
    W_attn = np.asarray(W_attn)
    W_proj = np.asarray(W_proj)
    bQ = np.asarray(bQ)
    bK = np.asarray(bK)
    for core in range(N_CORES):
        b, g = divmod(core, G)
        qbase = g * 256
        kbase = C + g * 256
        chunk_cols = [
            W_attn[:, qbase:qbase + 128],          # q01
            W_attn[:, kbase:kbase + 128],          # k01
            W_attn[:, qbase + 128:qbase + 256],    # q23
            W_attn[:, kbase + 128:kbase + 256],    # k23
        ]
        w_qk = np.stack([c.astype(bf).reshape(8, 128, 128).transpose(1, 0, 2)
                         for c in chunk_cols], axis=1)  # [128, 4, 8, 128]
        w_v = np.ascontiguousarray(
            W_attn[:, 2 * C + g * 256: 2 * C + (g + 1) * 256]
            .astype(bf).reshape(8, 128, 256).transpose(1, 0, 2))  # [128,8,256]
        w_p = np.ascontiguousarray(
            W_proj[g * 256:(g + 1) * 256, :]
            .astype(bf).reshape(2, 128, C).transpose(1, 0, 2))  # [128,2,1024]
        bias = np.zeros((128, 4), dtype=f)
        # chunk order q01, k01, q23, k23
        for ci, (src, pair) in enumerate(
                [(bQ, 0), (bK, 0), (bQ, 1), (bK, 1)]):
            h0 = g * HPG + pair * 2
            bias[0:64, ci] = src[h0]
            bias[64:128, ci] = src[h0 + 1]
        in_maps.append({
            "x_t": xtb[b],
            "w_qk": np.ascontiguousarray(w_qk),
            "w_v": w_v,
            "w_p": w_p,
            "cos_r": cos_r[b],
            "sin_r": sin_r[b],
            "bias_qk": bias,
            "tri": np.ascontiguousarray(tri),
        })
    return in_maps


def _get_nc(loop_k: int = 1):
    key = ("nc", loop_k)
    if key not in _cache:
        _install_waitsplit()
        _cache[key] = _build(loop_k)
    return _cache[key]


def run_spmd(in_maps):
    from concourse.bass_utils import run_bass_kernel_spmd

    nc = _get_nc()
    return run_bass_kernel_spmd(nc, in_maps, core_ids=list(range(N_CORES)))


def kernel(x, rope_cos, rope_sin, W_attn, b_attn, W_proj, b_proj, bQ, bK):
    in_maps = _prep_inputs(x, rope_cos, rope_sin, W_attn, b_attn, W_proj, b_proj,
                           bQ, bK)
    res = run_spmd(in_maps)
    outs = [res.results[c]["out"] for c in range(N_CORES)]
    b_proj = np.asarray(b_proj, dtype=np.float64)
    full = np.empty((B, T, C), dtype=np.float32)
    for b in range(B):
        acc = np.zeros((T, C), dtype=np.float64)
        for g in range(G):
            acc += outs[b * G + g].astype(np.float64)
        full[b] = (acc + b_proj).astype(np.float32)
    return full
